# revision 1
# baseline (speedup 1.0000x reference)
"""Trainium2 Bass kernel for nn_Deep_AD_F_58213986730479 (dense_cnn).

Math (per iteration t of 3):
    feats = 4 one-pixel zero-padded shifts (N,S,W,E) of x        [n,4,h,w]
    d     = conv3x3(feats, W[t]) + b[t]                          [n,4,h,w]
    x    -= sum_k d_k * exp(-d_k^2) / 4

Implementation:
  - Pure data parallel: batch 32 -> 8 cores x 4 images.
  - The shift+conv composes into a 21-tap stencil on x. Vertical taps are
    applied with banded-matrix matmuls on TensorE (contraction over image
    rows on partitions); horizontal taps via 5 column-shifted accumulating
    matmuls into PSUM. Boundary semantics of the double zero-padding are
    exact: row-edge terms fold into per-tile band-matrix variants; column
    edge terms are two N=1 correction matmuls per channel.
  - exp(-d^2) comes from one ScalarE op: Derivative_Erf = 2/sqrt(pi)*exp(-x^2);
    the bias add (d+b) rides free in the activation and in the DVE
    scalar_tensor_tensor that forms gated = (d+b)*e. Channel sum on GpSimd,
    final x update is one fused DVE scalar_tensor_tensor.
  - Each 512x512 image is 5 row-tiles [128,512] (stride 116, 6-row halo);
    3 iterations shrink the valid halo by 2 rows each, so no cross-tile
    traffic is ever needed.
"""
import sys

sys.path.insert(0, "/opt/trn_rl_repo")

import math
import numpy as np

import concourse.bass as bass
import concourse.bacc as bacc
import concourse.mybir as mybir
from concourse.tile import TileContext
from concourse.bass_utils import run_bass_kernel_spmd

F32 = mybir.dt.float32
F32R = mybir.dt.float32r
BF16 = mybir.dt.bfloat16
AF = mybir.ActivationFunctionType
ALU = mybir.AluOpType

NCORES = 8
IMGS = 4          # images per core
H = W_IMG = 512
T_ITERS = 3
KCH = 4
NTILES = 5
TSTART = [-6, 110, 226, 342, 458]   # image row held by partition 0 of tile j
CORE_LO = 6                          # first owned partition of each tile
CORE_ROWS = [116, 116, 116, 116, 48]
C_UPD = math.sqrt(math.pi) / 8.0     # 1/4 * sqrt(pi)/2 (Derivative_Erf scale)

# feats channel order in reference: N, S, W, E
OY = [-1, 1, 0, 0]
OX = [0, 0, -1, 1]

DXS = [0, -1, 1, -2, 2]

# debug bisect flags
_SKIP_CORR = False
_PSUM_BUFS = 2
_INPLACE_UPD = True
_MASK_AP = True
_TILE_SET = None  # e.g. [2] to restrict tiles (debug)
_PAD_BMAT = True
_MM_DTYPE = __import__("os").environ.get("KERNEL_MM_DTYPE", "f32")  # f32 | f32r | bf16
_SIMPLE_BIAS = False  # Dx=0 first: full-range start=True write


def _composite_taps(Wc):
    """T[t,k,Dy+2,Dx+2] = sum of W[t,k,i,dy+1,dx+1] with dy+oy_i=Dy, dx+ox_i=Dx."""
    taps = np.zeros((T_ITERS, KCH, 5, 5), np.float64)
    for t in range(T_ITERS):
        for k in range(KCH):
            for i in range(4):
                for dy in (-1, 0, 1):
                    for dx in (-1, 0, 1):
                        taps[t, k, dy + OY[i] + 2, dx + OX[i] + 2] += Wc[
                            t, k, i, dy + 1, dx + 1
                        ]
    return taps


def _build_bmats(Wc):
    """Dense lhsT matrices, returned as array [NB,128,128] f32 plus an index fn.

    Layout per (t,k): 5 generic B_Dx, then 3 top-variant (Dx=-1,0,1), then
    3 bottom-variant, then left corr, right corr = 13 matrices.
    B[in_row, out_row] = tap[in-out, Dx].
    """
    taps = _composite_taps(Wc)
    mats = []
    index = {}

    def band(vals_by_dy):
        B = np.zeros((128, 128), np.float64)
        for dy, v in vals_by_dy.items():
            B += v * np.eye(128, k=-dy)
        return B

    for t in range(T_ITERS):
        for k in range(KCH):
            per_dx = {}
            for Dx in (-2, -1, 0, 1, 2):
                per_dx[Dx] = band(
                    {Dy: taps[t, k, Dy + 2, Dx + 2] for Dy in range(-2, 3)}
                )
            for Dx in DXS:
                index[(t, k, Dx, "mid")] = len(mats)
                mats.append(per_dx[Dx])
            for Dx in (-1, 0, 1):
                Btop = per_dx[Dx].copy()
                # image row 0 = partition CORE_LO of tile 0: remove south-ch dy=-1
                Btop[CORE_LO, CORE_LO] -= Wc[t, k, 1, 0, Dx + 1]
                index[(t, k, Dx, "top")] = len(mats)
                mats.append(Btop)
            for Dx in (-1, 0, 1):
                Bbot = per_dx[Dx].copy()
                # image row 511 = partition 53 of tile 4: remove north-ch dy=+1
                p = CORE_LO + (H - 1) - TSTART[4] - CORE_LO  # = 53
                Bbot[p, p] -= Wc[t, k, 0, 2, Dx + 1]
                index[(t, k, Dx, "bot")] = len(mats)
                mats.append(Bbot)
            # column-edge corrections (vertical 3-tap bands)
            Bl = band({dy: -Wc[t, k, 3, dy + 1, 0] for dy in (-1, 0, 1)})
            index[(t, k, "corrL")] = len(mats)
            mats.append(Bl)
            Br = band({dy: -Wc[t, k, 2, dy + 1, 2] for dy in (-1, 0, 1)})
            index[(t, k, "corrR")] = len(mats)
            mats.append(Br)
    arr = np.stack(mats).astype(np.float32)
    return arr, index


_NB = T_ITERS * KCH * 13


def _build_masks():
    """Per-tile update masks [128, NTILES]: -C_UPD at real image rows, 0 at pad."""
    m = np.full((128, NTILES), -C_UPD, np.float32)
    for j in range(NTILES):
        r0 = TSTART[j]
        plo = max(0, -r0)
        phi = min(128, H - r0)
        m[0:plo, j] = 0.0
        m[phi:128, j] = 0.0
    return m


def _build_nc(bvals, bindex):
    nc = bacc.Bacc(None, target_bir_lowering=False)
    xs = nc.declare_dram_parameter("xs", [IMGS, H, W_IMG], F32, isOutput=False)
    bmw = _NB * 128 + (0 if _MM_DTYPE == "bf16" else NTILES)
    if _PAD_BMAT:
        bmw = (bmw + 127) // 128 * 128
    bdt = BF16 if _MM_DTYPE == "bf16" else F32
    bm = nc.declare_dram_parameter("bmat", [128, bmw], bdt, isOutput=False)
    if _MM_DTYPE == "bf16":
        aux = nc.declare_dram_parameter("aux", [128, 16], F32, isOutput=False)
    yo = nc.declare_dram_parameter("out", [IMGS, H, W_IMG], F32, isOutput=True)

    with TileContext(nc) as tc:
        with (
            tc.tile_pool(name="wts", bufs=1) as wp,
            tc.tile_pool(name="xdata", bufs=1) as xp,
            tc.tile_pool(name="work", bufs=2) as sp,
            tc.tile_pool(name="ps", bufs=2, space="PSUM") as pp,
        ):
            bmt = wp.tile([128, bmw], bdt, tag="bmt")
            pert = KCH * 13 * 128  # cols per iteration t
            for t in range(T_ITERS):
                nc.sync.dma_start(
                    out=bmt[:, t * pert : (t + 1) * pert],
                    in_=bm[:, t * pert : (t + 1) * pert],
                )
            if bmw > T_ITERS * pert:
                nc.sync.dma_start(
                    out=bmt[:, T_ITERS * pert :], in_=bm[:, T_ITERS * pert :]
                )
            if _MM_DTYPE == "bf16":
                auxt = wp.tile([128, 16], F32, tag="auxt")
                nc.sync.dma_start(out=auxt[:], in_=aux[:])

            if _SIMPLE_BIAS:
                bias_tiles = {}
                for t in range(T_ITERS):
                    for k in range(KCH):
                        bb = wp.tile([128, 1], F32, tag=f"bias{t}_{k}")
                        nc.vector.memset(bb[:], float(bvals[t, k]))
                        bias_tiles[t, k] = bb
            else:
                bias_t = wp.tile([128, T_ITERS * KCH], F32, tag="bias")
                for t in range(T_ITERS):
                    for k in range(KCH):
                        nc.vector.memset(
                            bias_t[:, t * KCH + k : t * KCH + k + 1], float(bvals[t, k])
                        )

            def bmat(idx):
                ap = bmt[:, idx * 128 : (idx + 1) * 128]
                if _MM_DTYPE == "f32r":
                    ap = ap.bitcast(F32R)
                return ap

            def mm_rhs(ap):
                if _MM_DTYPE == "f32r":
                    return ap.bitcast(F32R)
                return ap

            tset = _TILE_SET if _TILE_SET is not None else list(range(NTILES))
            xt = {}
            for im in range(IMGS):
                for j in tset:
                    tile = xp.tile([128, W_IMG], F32, tag=f"x{im}_{j}")
                    xt[im, j] = tile
                    r0 = TSTART[j]
                    plo = max(0, -r0)
                    phi = min(128, H - r0)
                    if plo > 0 or phi < 128:
                        nc.vector.memset(tile[:], 0.0)
                    nc.sync.dma_start(
                        out=tile[plo:phi, :], in_=xs[im, r0 + plo : r0 + phi, :]
                    )

            for it in range(T_ITERS):
                for im in range(IMGS):
                    for j in tset:
                        x_t = xt[im, j]
                        cls = "top" if j == 0 else ("bot" if j == NTILES - 1 else "mid")
                        if _MM_DTYPE == "bf16":
                            xmm = sp.tile([128, W_IMG], BF16, tag="xb")
                            nc.scalar.copy(xmm[:], x_t[:])
                        else:
                            xmm = x_t
                        dks = []
                        for k in range(KCH):
                            dk_t = pp.tile([128, W_IMG], F32, tag=f"d{k}")
                            dks.append(dk_t)
                        for k in range(KCH):
                            base = 0
                            d = dks[k]
                            for Dx in DXS:
                                key = (
                                    (it, k, Dx, cls)
                                    if (it, k, Dx, cls) in bindex
                                    else (it, k, Dx, "mid")
                                )
                                ocl = max(0, -Dx)
                                och = W_IMG - max(0, Dx)
                                nc.tensor.matmul(
                                    d[:, base + ocl : base + och],
                                    bmat(bindex[key]),
                                    mm_rhs(xmm[:, ocl + Dx : och + Dx]),
                                    start=(Dx == 0),
                                    stop=False,
                                )
                            if not _SKIP_CORR:
                                nc.tensor.matmul(
                                    d[:, base : base + 1],
                                    bmat(bindex[(it, k, "corrL")]),
                                    mm_rhs(xmm[:, 0:1]),
                                    start=False,
                                    stop=False,
                                )
                                nc.tensor.matmul(
                                    d[:, base + W_IMG - 1 : base + W_IMG],
                                    bmat(bindex[(it, k, "corrR")]),
                                    mm_rhs(xmm[:, W_IMG - 1 : W_IMG]),
                                    start=False,
                                    stop=True,
                                )
                        g = sp.tile([128, KCH * W_IMG], F32, tag="g")
                        for k in range(KCH):
                            base = k * W_IMG
                            ek = sp.tile([128, W_IMG], F32, tag=f"e{k}")
                            nc.scalar.activation(
                                ek[:],
                                dks[k][:],
                                AF.Derivative_Erf,
                                bias=(bias_tiles[it, k][:, 0:1] if _SIMPLE_BIAS
                                      else bias_t[:, it * KCH + k : it * KCH + k + 1]),
                                scale=1.0,
                            )
                            nc.vector.scalar_tensor_tensor(
                                out=g[:, base : base + W_IMG],
                                in0=dks[k][:],
                                scalar=float(bvals[it, k]),
                                in1=ek[:],
                                op0=ALU.add,
                                op1=ALU.mult,
                            )
                        s01 = sp.tile([128, W_IMG], F32, tag="s01")
                        s23 = sp.tile([128, W_IMG], F32, tag="s23")
                        stot = sp.tile([128, W_IMG], F32, tag="stot")
                        nc.gpsimd.tensor_tensor(
                            out=s01[:], in0=g[:, 0:512], in1=g[:, 512:1024], op=ALU.add
                        )
                        nc.gpsimd.tensor_tensor(
                            out=s23[:], in0=g[:, 1024:1536], in1=g[:, 1536:2048],
                            op=ALU.add,
                        )
                        nc.gpsimd.tensor_tensor(
                            out=stot[:], in0=s01[:], in1=s23[:], op=ALU.add
                        )
                        if _MM_DTYPE == "bf16":
                            mask_ap = auxt[:, j : j + 1] if _MASK_AP else -C_UPD
                        else:
                            mask_ap = (
                                bmt[:, _NB * 128 + j : _NB * 128 + j + 1]
                                if _MASK_AP
                                else -C_UPD
                            )
                        if _INPLACE_UPD:
                            nc.vector.scalar_tensor_tensor(
                                out=x_t[:],
                                in0=stot[:],
                                scalar=mask_ap,
                                in1=x_t[:],
                                op0=ALU.mult,
                                op1=ALU.add,
                            )
                        else:
                            x_new = xp.tile([128, W_IMG], F32, tag=f"xn{im}_{j}_{it}")
                            nc.vector.scalar_tensor_tensor(
                                out=x_new[:],
                                in0=stot[:],
                                scalar=mask_ap,
                                in1=x_t[:],
                                op0=ALU.mult,
                                op1=ALU.add,
                            )
                            xt[im, j] = x_new

            for im in range(IMGS):
                for j in tset:
                    rows = CORE_ROWS[j]
                    nc.sync.dma_start(
                        out=yo[im, 116 * j : 116 * j + rows, :],
                        in_=xt[im, j][CORE_LO : CORE_LO + rows, :],
                    )
    nc.compile()
    return nc


_CACHE = {}


def _get_program(Wc, bc):
    key = (Wc.tobytes(), bc.tobytes())
    if key not in _CACHE:
        barr, bindex = _build_bmats(Wc.astype(np.float64))
        # SBUF layout [p, n*128+m]
        if _MM_DTYPE == "bf16":
            parts = [barr.transpose(1, 0, 2).reshape(128, _NB * 128)]
            w0 = _NB * 128
        else:
            parts = [barr.transpose(1, 0, 2).reshape(128, _NB * 128), _build_masks()]
            w0 = _NB * 128 + NTILES
        if _PAD_BMAT:
            wpad = (w0 + 127) // 128 * 128 - w0
            if wpad:
                parts.append(np.zeros((128, wpad), np.float32))
        bflat = np.ascontiguousarray(np.concatenate(parts, axis=1), dtype=np.float32)
        if _MM_DTYPE == "bf16":
            import ml_dtypes

            bflat = bflat.astype(ml_dtypes.bfloat16)
        nc = _build_nc(bc.astype(np.float64), bindex)
        _CACHE[key] = (nc, bflat)
    return _CACHE[key]


def _install_trace_shim():
    """The agent image lacks antenv.axon_hooks; rebuild the NTFF hook from
    trn_boot's ctypes recipe and skip the artifact upload."""
    import types

    if "antenv.axon_hooks" in sys.modules:
        return
    try:
        from trn_agent_boot.trn_boot import _ntff_profile_via_ctypes

        hook = _ntff_profile_via_ctypes("/opt/axon/libaxon_pjrt.so")
    except Exception:
        hook = None
    mod = types.ModuleType("antenv.axon_hooks")
    mod.get_axon_ntff_profile_hook = lambda: hook
    mod.set_axon_ntff_profile_hook = lambda h: None
    sys.modules["antenv.axon_hooks"] = mod
    import concourse.bass_utils as bu

    bu.upload_artifacts = lambda d: "local://skipped"


def kernel(x, W, b, _trace=False, _tracedir=None):
    x = np.asarray(x)
    W = np.asarray(W)
    b = np.asarray(b)
    nc, bflat = _get_program(W, b)
    in_maps = []
    for c in range(NCORES):
        shard = np.ascontiguousarray(x[c * IMGS : (c + 1) * IMGS, 0]).astype(np.float32)
        im_map = {"xs": shard, "bmat": bflat}
        if _MM_DTYPE == "bf16":
            am = np.zeros((128, 16), np.float32)
            am[:, :NTILES] = _build_masks()
            im_map["aux"] = am
        in_maps.append(im_map)
    kw = {}
    if _trace:
        _install_trace_shim()
        kw = {"trace": True, "tmpdir": _tracedir}
    res = run_bass_kernel_spmd(nc, in_maps, list(range(NCORES)), **kw)
    out = np.concatenate([res.results[c]["out"] for c in range(NCORES)], axis=0)
    out = out[:, None].astype(x.dtype)
    kernel._last = res
    return out



# revision 17
# speedup vs baseline: 2.5751x; 2.5751x over previous
"""Trainium2 Bass kernel for nn_Deep_AD_F_58213986730479 (dense_cnn).

Math (per iteration t of 3):
    feats = 4 one-pixel zero-padded shifts (N,S,W,E) of x        [n,4,h,w]
    d     = conv3x3(feats, W[t]) + b[t]                          [n,4,h,w]
    x    -= sum_k d_k * exp(-d_k^2) / 4

Implementation:
  - Pure data parallel: batch 32 -> 8 cores x 4 images.
  - The shift+conv composes into a 21-tap stencil on x. Vertical taps are
    applied with banded-matrix matmuls on TensorE (contraction over image
    rows on partitions); horizontal taps via 5 column-shifted accumulating
    matmuls into PSUM. All matmuls run in float32r (1 cycle/row vs 4 for
    f32). f32r ISA restrictions (even moving counts, 8B-aligned dst) are
    met by padding x tiles to 516 columns with DRAM-staged zeros so every
    Dx pass is a full 512-wide matmul, and by expressing the two column-
    edge corrections as 2-col matmuls whose second lane reads a zero pad
    column via a negative-stride rhs AP.
  - exp(-d^2) comes from one ScalarE op: Derivative_Erf = 2/sqrt(pi)*exp(-x^2);
    the bias add (d+b) rides free in the activation and in the DVE
    scalar_tensor_tensor that forms gated = (d+b)*e. Channel sum on GpSimd,
    final x update is one fused DVE scalar_tensor_tensor (which rounds its
    output to f32r as the BIR verifier requires for f32r-matmul inputs).
  - Each 512x512 image is 5 row-tiles [128,516] (stride 116, 6-row halo);
    3 iterations shrink the valid halo by 2 rows each, so no cross-tile
    traffic is ever needed. Zero pad rows are staged in DRAM (no memsets).
  - Loop order (it, j, k, pass, im) keeps one stationary band matrix for 4
    consecutive matmuls and lets tile j's output DMA overlap tile j+1.
"""
import sys

sys.path.insert(0, "/opt/trn_rl_repo")

import math
import numpy as np

import concourse.bass as bass
import concourse.bacc as bacc
import concourse.mybir as mybir
from concourse.tile import TileContext
from concourse.bass_utils import run_bass_kernel_spmd

F32 = mybir.dt.float32
F32R = mybir.dt.float32r
AF = mybir.ActivationFunctionType
ALU = mybir.AluOpType

NCORES = 8
IMGS = 4          # images per core
H = WI = 512
XW = 516          # x tile width: 2 zero pad cols each side
XROWS = 592       # padded DRAM rows: 6 zero, 512 image, 74 zero
T_ITERS = 3
KCH = 4
NTILES = 5
TSTART = [-6, 110, 226, 342, 458]   # image row held by partition 0 of tile j
CORE_LO = 6                          # first owned partition of each tile
CORE_ROWS = [116, 116, 116, 116, 48]
C_UPD = math.sqrt(math.pi) / 8.0     # 1/4 * sqrt(pi)/2 (Derivative_Erf scale)

# feats channel order in reference: N, S, W, E
OY = [-1, 1, 0, 0]
OX = [0, 0, -1, 1]

DXS = [0, -1, 1, -2, 2]

_MM_DTYPE = __import__("os").environ.get("KERNEL_MM_DTYPE", "f32r")  # f32 | f32r


def _composite_taps(Wc):
    """T[t,k,Dy+2,Dx+2] = sum of W[t,k,i,dy+1,dx+1] with dy+oy_i=Dy, dx+ox_i=Dx."""
    taps = np.zeros((T_ITERS, KCH, 5, 5), np.float64)
    for t in range(T_ITERS):
        for k in range(KCH):
            for i in range(4):
                for dy in (-1, 0, 1):
                    for dx in (-1, 0, 1):
                        taps[t, k, dy + OY[i] + 2, dx + OX[i] + 2] += Wc[
                            t, k, i, dy + 1, dx + 1
                        ]
    return taps


def _build_bmats(Wc):
    """Dense lhsT matrices, returned as array [NB,128,128] f32 plus an index fn.

    Layout per (t,k): 5 generic B_Dx, then 3 top-variant (Dx=-1,0,1), then
    3 bottom-variant, then left corr, right corr = 13 matrices.
    B[in_row, out_row] = tap[in-out, Dx].
    """
    taps = _composite_taps(Wc)
    mats = []
    index = {}

    def band(vals_by_dy):
        B = np.zeros((128, 128), np.float64)
        for dy, v in vals_by_dy.items():
            B += v * np.eye(128, k=-dy)
        return B

    for t in range(T_ITERS):
        for k in range(KCH):
            per_dx = {}
            for Dx in (-2, -1, 0, 1, 2):
                per_dx[Dx] = band(
                    {Dy: taps[t, k, Dy + 2, Dx + 2] for Dy in range(-2, 3)}
                )
            for Dx in DXS:
                index[(t, k, Dx, "mid")] = len(mats)
                mats.append(per_dx[Dx])
            for Dx in (-1, 0, 1):
                Btop = per_dx[Dx].copy()
                # image row 0 = partition CORE_LO of tile 0: remove south-ch dy=-1
                Btop[CORE_LO, CORE_LO] -= Wc[t, k, 1, 0, Dx + 1]
                index[(t, k, Dx, "top")] = len(mats)
                mats.append(Btop)
            for Dx in (-1, 0, 1):
                Bbot = per_dx[Dx].copy()
                # image row 511 = partition 53 of tile 4: remove north-ch dy=+1
                p = (H - 1) - TSTART[4]  # = 53
                Bbot[p, p] -= Wc[t, k, 0, 2, Dx + 1]
                index[(t, k, Dx, "bot")] = len(mats)
                mats.append(Bbot)
            # column-edge corrections (vertical 3-tap bands)
            Bl = band({dy: -Wc[t, k, 3, dy + 1, 0] for dy in (-1, 0, 1)})
            index[(t, k, "corrL")] = len(mats)
            mats.append(Bl)
            Br = band({dy: -Wc[t, k, 2, dy + 1, 2] for dy in (-1, 0, 1)})
            index[(t, k, "corrR")] = len(mats)
            mats.append(Br)
    arr = np.stack(mats).astype(np.float32)
    return arr, index


_NB = T_ITERS * KCH * 13
_PERTK = 13 * 128  # bmat cols per (t,k)


def _build_masks():
    """Per-tile update masks [128, NTILES]: -C_UPD at real image rows, 0 at pad."""
    m = np.full((128, NTILES), -C_UPD, np.float32)
    for j in range(NTILES):
        r0 = TSTART[j]
        plo = max(0, -r0)
        phi = min(128, H - r0)
        m[0:plo, j] = 0.0
        m[phi:128, j] = 0.0
    return m


def _build_nc(bvals, bindex):
    nc = bacc.Bacc(None, target_bir_lowering=False)
    xs = nc.declare_dram_parameter("xs", [IMGS, XROWS, XW], F32, isOutput=False)
    bmw = _NB * 128
    bm = nc.declare_dram_parameter("bmat", [128, bmw], F32, isOutput=False)
    aux = nc.declare_dram_parameter("aux", [128, 16], F32, isOutput=False)
    yo = nc.declare_dram_parameter("out", [IMGS, H, WI], F32, isOutput=True)

    mm_dt = F32R if _MM_DTYPE == "f32r" else F32

    def as_mm(ap):
        # DRAM-side AP relabel so DMA into an f32r tile is cast-free
        return ap.bitcast(F32R) if _MM_DTYPE == "f32r" else ap

    def as_f32(ap):
        # non-PE engines and DMA-out read f32r tiles as plain f32
        return ap.bitcast(F32) if _MM_DTYPE == "f32r" else ap

    with TileContext(nc) as tc:
        with (
            tc.tile_pool(name="wts", bufs=1) as wp,
            tc.tile_pool(name="xdata", bufs=1) as xp,
            tc.tile_pool(name="work", bufs=2) as sp,
            tc.tile_pool(name="ps", bufs=2, space="PSUM") as pp,
        ):
            bmt = wp.tile([128, bmw], mm_dt, tag="bmt")
            # fine-grained weight loads so the first matmul starts early
            for t in range(T_ITERS):
                for k in range(KCH):
                    i0 = (t * KCH + k) * _PERTK
                    nc.sync.dma_start(
                        out=bmt[:, i0 : i0 + _PERTK],
                        in_=as_mm(bm[:, i0 : i0 + _PERTK]),
                    )
            auxt = wp.tile([128, 16], F32, tag="auxt")
            nc.sync.dma_start(out=auxt[:], in_=aux[:])

            bias_t = wp.tile([128, T_ITERS * KCH], F32, tag="bias")
            for t in range(T_ITERS):
                for k in range(KCH):
                    nc.vector.memset(
                        bias_t[:, t * KCH + k : t * KCH + k + 1], float(bvals[t, k])
                    )

            def bmat(idx):
                return bmt[:, idx * 128 : (idx + 1) * 128]

            xt = {}
            for im in range(IMGS):
                for j in range(NTILES):
                    tile = xp.tile([128, XW], mm_dt, tag=f"x{im}_{j}")
                    xt[im, j] = tile
                    nc.sync.dma_start(
                        out=tile[:],
                        in_=as_mm(xs[im, TSTART[j] + 6 : TSTART[j] + 134, :]),
                    )

            for it in range(T_ITERS):
                for j in range(NTILES):
                    cls = "top" if j == 0 else ("bot" if j == NTILES - 1 else "mid")
                    # 3 gated slots A/B/C per image; GpSimd folds them in
                    # place so no extra sum tiles are needed: A=g0, B=g1, A+=B;
                    # B=g2, C=g3, B+=C, A+=B; update reads A.
                    g = {
                        im: sp.tile([128, 3 * WI], F32, tag=f"g{im}", name=f"g{im}")
                        for im in range(IMGS)
                    }
                    for k in range(KCH):
                        dks = {
                            im: pp.tile([128, WI], F32, tag=f"d{im}", name=f"d{im}")
                            for im in range(IMGS)
                        }
                        passes = []
                        for Dx in DXS:
                            key = (
                                (it, k, Dx, cls)
                                if (it, k, Dx, cls) in bindex
                                else (it, k, Dx, "mid")
                            )
                            # out image col c reads xpad col c+2+Dx
                            passes.append(
                                (bindex[key], 0, WI, slice(2 + Dx, 514 + Dx))
                            )
                        # corrL: out cols (0,1) <- (x col 0, zero pad col 1)
                        passes.append(
                            (bindex[(it, k, "corrL")], 0, 2, slice(2, 0, -1))
                        )
                        # corrR: out cols (510,511) <- (zero pad col 514, x col 511)
                        passes.append(
                            (bindex[(it, k, "corrR")], 510, 512, slice(514, 512, -1))
                        )
                        for pi, (bi, o0, o1, rsl) in enumerate(passes):
                            for im in range(IMGS):
                                nc.tensor.matmul(
                                    dks[im][:, o0:o1],
                                    bmat(bi),
                                    xt[im, j][:, rsl],
                                    start=(pi == 0),
                                    stop=(pi == len(passes) - 1),
                                )
                        slot = [0, 1, 1, 2][k] * WI  # A,B,B,C
                        for im in range(IMGS):
                            ek = sp.tile([128, WI], F32, tag=f"e{im}")
                            nc.scalar.activation(
                                ek[:],
                                dks[im][:],
                                AF.Derivative_Erf,
                                bias=bias_t[:, it * KCH + k : it * KCH + k + 1],
                                scale=1.0,
                            )
                            nc.vector.scalar_tensor_tensor(
                                out=g[im][:, slot : slot + WI],
                                in0=dks[im][:],
                                scalar=float(bvals[it, k]),
                                in1=ek[:],
                                op0=ALU.add,
                                op1=ALU.mult,
                            )
                        if k == 1:
                            for im in range(IMGS):
                                nc.gpsimd.tensor_tensor(
                                    out=g[im][:, 0:WI],
                                    in0=g[im][:, 0:WI],
                                    in1=g[im][:, WI : 2 * WI],
                                    op=ALU.add,
                                )
                        if k == 3:
                            for im in range(IMGS):
                                nc.gpsimd.tensor_tensor(
                                    out=g[im][:, WI : 2 * WI],
                                    in0=g[im][:, WI : 2 * WI],
                                    in1=g[im][:, 2 * WI : 3 * WI],
                                    op=ALU.add,
                                )
                    for im in range(IMGS):
                        nc.gpsimd.tensor_tensor(
                            out=g[im][:, 0:WI],
                            in0=g[im][:, 0:WI],
                            in1=g[im][:, WI : 2 * WI],
                            op=ALU.add,
                        )
                        # out keeps the tile's native (f32r) dtype so the DVE
                        # rounds — required for f32r-matmul inputs
                        nc.vector.scalar_tensor_tensor(
                            out=xt[im, j][:, 2:514],
                            in0=g[im][:, 0:WI],
                            scalar=auxt[:, j : j + 1],
                            in1=as_f32(xt[im, j][:, 2:514]),
                            op0=ALU.mult,
                            op1=ALU.add,
                        )
                        if it == T_ITERS - 1:
                            rows = CORE_ROWS[j]
                            nc.sync.dma_start(
                                out=yo[im, 116 * j : 116 * j + rows, :],
                                in_=as_f32(
                                    xt[im, j][CORE_LO : CORE_LO + rows, 2:514]
                                ),
                            )
    nc.compile()
    return nc


_CACHE = {}


def _get_program(Wc, bc):
    key = (Wc.tobytes(), bc.tobytes())
    if key not in _CACHE:
        barr, bindex = _build_bmats(Wc.astype(np.float64))
        # SBUF layout [p, n*128+m]
        bflat = np.ascontiguousarray(
            barr.transpose(1, 0, 2).reshape(128, _NB * 128), dtype=np.float32
        )
        nc = _build_nc(bc.astype(np.float64), bindex)
        _CACHE[key] = (nc, bflat)
    return _CACHE[key]


def _install_trace_shim():
    """The agent image lacks antenv.axon_hooks; rebuild the NTFF hook from
    trn_boot's ctypes recipe and skip the artifact upload."""
    import types

    if "antenv.axon_hooks" in sys.modules:
        return
    try:
        from trn_agent_boot.trn_boot import _ntff_profile_via_ctypes

        hook = _ntff_profile_via_ctypes("/opt/axon/libaxon_pjrt.so")
    except Exception:
        hook = None
    mod = types.ModuleType("antenv.axon_hooks")
    mod.get_axon_ntff_profile_hook = lambda: hook
    mod.set_axon_ntff_profile_hook = lambda h: None
    sys.modules["antenv.axon_hooks"] = mod
    import concourse.bass_utils as bu

    bu.upload_artifacts = lambda d: "local://skipped"


def kernel(x, W, b, _trace=False, _tracedir=None):
    x = np.asarray(x)
    W = np.asarray(W)
    b = np.asarray(b)
    nc, bflat = _get_program(W, b)
    am = np.zeros((128, 16), np.float32)
    am[:, :NTILES] = _build_masks()
    in_maps = []
    for c in range(NCORES):
        shard = np.zeros((IMGS, XROWS, XW), np.float32)
        shard[:, 6 : 6 + H, 2 : 2 + WI] = x[c * IMGS : (c + 1) * IMGS, 0]
        in_maps.append({"xs": shard, "bmat": bflat, "aux": am})
    kw = {}
    if _trace:
        _install_trace_shim()
        kw = {"trace": True, "tmpdir": _tracedir}
    res = run_bass_kernel_spmd(nc, in_maps, list(range(NCORES)), **kw)
    out = np.concatenate([res.results[c]["out"] for c in range(NCORES)], axis=0)
    out = out[:, None].astype(x.dtype)
    kernel._last = res
    return out


# revision 19
# speedup vs baseline: 2.7312x; 1.0606x over previous
"""Trainium2 Bass kernel for nn_Deep_AD_F_58213986730479 (dense_cnn).

Math (per iteration t of 3):
    feats = 4 one-pixel zero-padded shifts (N,S,W,E) of x        [n,4,h,w]
    d     = conv3x3(feats, W[t]) + b[t]                          [n,4,h,w]
    x    -= sum_k d_k * exp(-d_k^2) / 4

Implementation:
  - Pure data parallel: batch 32 -> 8 cores x 4 images.
  - The shift+conv composes into a 21-tap stencil on x. Vertical taps are
    applied with banded-matrix matmuls on TensorE (contraction over image
    rows on partitions); horizontal taps via 5 column-shifted accumulating
    matmuls into PSUM. All matmuls run in float32r (1 cycle/row vs 4 for
    f32). f32r ISA restrictions (even moving counts, 8B-aligned dst) are
    met by padding x tiles to 516 columns with DRAM-staged zeros so every
    Dx pass is a full 512-wide matmul, and by expressing the two column-
    edge corrections as 2-col matmuls whose second lane reads a zero pad
    column via a negative-stride rhs AP.
  - exp(-d^2) comes from one ScalarE op: Derivative_Erf = 2/sqrt(pi)*exp(-x^2);
    the bias add (d+b) rides free in the activation and in the DVE
    scalar_tensor_tensor that forms gated = (d+b)*e. Channel sum on GpSimd,
    final x update is one fused DVE scalar_tensor_tensor (which rounds its
    output to f32r as the BIR verifier requires for f32r-matmul inputs).
  - Each 512x512 image is 5 row-tiles [128,516] (stride 116, 6-row halo);
    3 iterations shrink the valid halo by 2 rows each, so no cross-tile
    traffic is ever needed. Zero pad rows are staged in DRAM (no memsets).
  - Loop order (it, j, k, pass, im) keeps one stationary band matrix for 4
    consecutive matmuls and lets tile j's output DMA overlap tile j+1.
"""
import sys

sys.path.insert(0, "/opt/trn_rl_repo")

import math
import numpy as np

import concourse.bass as bass
import concourse.bacc as bacc
import concourse.mybir as mybir
from concourse.tile import TileContext
from concourse.bass_utils import run_bass_kernel_spmd

F32 = mybir.dt.float32
F32R = mybir.dt.float32r
AF = mybir.ActivationFunctionType
ALU = mybir.AluOpType

NCORES = 8
IMGS = 4          # images per core
H = WI = 512
XW = 516          # x tile width: 2 zero pad cols each side
XROWS = 592       # padded DRAM rows: 6 zero, 512 image, 74 zero
T_ITERS = 3
KCH = 4
NTILES = 5
TSTART = [-6, 110, 226, 342, 458]   # image row held by partition 0 of tile j
CORE_LO = 6                          # first owned partition of each tile
CORE_ROWS = [116, 116, 116, 116, 48]
C_UPD = math.sqrt(math.pi) / 8.0     # 1/4 * sqrt(pi)/2 (Derivative_Erf scale)

# feats channel order in reference: N, S, W, E
OY = [-1, 1, 0, 0]
OX = [0, 0, -1, 1]

DXS = [0, -1, 1, -2, 2]

_MM_DTYPE = __import__("os").environ.get("KERNEL_MM_DTYPE", "f32r")  # f32 | f32r


def _composite_taps(Wc):
    """T[t,k,Dy+2,Dx+2] = sum of W[t,k,i,dy+1,dx+1] with dy+oy_i=Dy, dx+ox_i=Dx."""
    taps = np.zeros((T_ITERS, KCH, 5, 5), np.float64)
    for t in range(T_ITERS):
        for k in range(KCH):
            for i in range(4):
                for dy in (-1, 0, 1):
                    for dx in (-1, 0, 1):
                        taps[t, k, dy + OY[i] + 2, dx + OX[i] + 2] += Wc[
                            t, k, i, dy + 1, dx + 1
                        ]
    return taps


def _build_bmats(Wc):
    """Dense lhsT matrices, returned as array [NB,128,128] f32 plus an index fn.

    Layout per (t,k): 5 generic B_Dx, then 3 top-variant (Dx=-1,0,1), then
    3 bottom-variant, then left corr, right corr = 13 matrices.
    B[in_row, out_row] = tap[in-out, Dx].
    """
    taps = _composite_taps(Wc)
    mats = []
    index = {}

    def band(vals_by_dy):
        B = np.zeros((128, 128), np.float64)
        for dy, v in vals_by_dy.items():
            B += v * np.eye(128, k=-dy)
        return B

    for t in range(T_ITERS):
        for k in range(KCH):
            per_dx = {}
            for Dx in (-2, -1, 0, 1, 2):
                per_dx[Dx] = band(
                    {Dy: taps[t, k, Dy + 2, Dx + 2] for Dy in range(-2, 3)}
                )
            for Dx in DXS:
                index[(t, k, Dx, "mid")] = len(mats)
                mats.append(per_dx[Dx])
            for Dx in (-1, 0, 1):
                Btop = per_dx[Dx].copy()
                # image row 0 = partition CORE_LO of tile 0: remove south-ch dy=-1
                Btop[CORE_LO, CORE_LO] -= Wc[t, k, 1, 0, Dx + 1]
                index[(t, k, Dx, "top")] = len(mats)
                mats.append(Btop)
            for Dx in (-1, 0, 1):
                Bbot = per_dx[Dx].copy()
                # image row 511 = partition 53 of tile 4: remove north-ch dy=+1
                p = (H - 1) - TSTART[4]  # = 53
                Bbot[p, p] -= Wc[t, k, 0, 2, Dx + 1]
                index[(t, k, Dx, "bot")] = len(mats)
                mats.append(Bbot)
            # column-edge corrections (vertical 3-tap bands)
            Bl = band({dy: -Wc[t, k, 3, dy + 1, 0] for dy in (-1, 0, 1)})
            index[(t, k, "corrL")] = len(mats)
            mats.append(Bl)
            Br = band({dy: -Wc[t, k, 2, dy + 1, 2] for dy in (-1, 0, 1)})
            index[(t, k, "corrR")] = len(mats)
            mats.append(Br)
    arr = np.stack(mats).astype(np.float32)
    return arr, index


_NB = T_ITERS * KCH * 13
_PERTK = 13 * 128  # bmat cols per (t,k)


def _build_masks():
    """Per-tile update masks [128, NTILES]: -C_UPD at real image rows, 0 at pad."""
    m = np.full((128, NTILES), -C_UPD, np.float32)
    for j in range(NTILES):
        r0 = TSTART[j]
        plo = max(0, -r0)
        phi = min(128, H - r0)
        m[0:plo, j] = 0.0
        m[phi:128, j] = 0.0
    return m


def _build_nc(bvals, bindex):
    nc = bacc.Bacc(None, target_bir_lowering=False)
    xs = nc.declare_dram_parameter("xs", [IMGS, XROWS, XW], F32, isOutput=False)
    bmw = _NB * 128
    bm = nc.declare_dram_parameter("bmat", [128, bmw], F32, isOutput=False)
    aux = nc.declare_dram_parameter("aux", [128, 16], F32, isOutput=False)
    yo = nc.declare_dram_parameter("out", [IMGS, H, WI], F32, isOutput=True)

    mm_dt = F32R if _MM_DTYPE == "f32r" else F32

    def as_mm(ap):
        # DRAM-side AP relabel so DMA into an f32r tile is cast-free
        return ap.bitcast(F32R) if _MM_DTYPE == "f32r" else ap

    def as_f32(ap):
        # non-PE engines and DMA-out read f32r tiles as plain f32
        return ap.bitcast(F32) if _MM_DTYPE == "f32r" else ap

    with TileContext(nc) as tc:
        with (
            tc.tile_pool(name="wts", bufs=1) as wp,
            tc.tile_pool(name="xdata", bufs=1) as xp,
            tc.tile_pool(name="work", bufs=2) as sp,
            tc.tile_pool(name="ps", bufs=2, space="PSUM") as pp,
        ):
            bmt = wp.tile([128, bmw], mm_dt, tag="bmt")

            def load_bmt(t, k):
                i0 = (t * KCH + k) * _PERTK
                nc.sync.dma_start(
                    out=bmt[:, i0 : i0 + _PERTK],
                    in_=as_mm(bm[:, i0 : i0 + _PERTK]),
                )

            xt = {}

            def load_x(j):
                for im in range(IMGS):
                    tile = xp.tile(
                        [128, XW], mm_dt, tag=f"x{im}_{j}", name=f"x{im}_{j}"
                    )
                    xt[im, j] = tile
                    nc.sync.dma_start(
                        out=tile[:],
                        in_=as_mm(xs[im, TSTART[j] + 6 : TSTART[j] + 134, :]),
                    )

            # DMA order matters: the first tile group needs bmt(t0,k0) and
            # x(*,0) first; everything else streams in behind while the PE
            # is already busy.
            load_bmt(0, 0)
            load_x(0)
            for k in range(1, KCH):
                load_bmt(0, k)
            for j in range(1, NTILES):
                load_x(j)
            for t in range(1, T_ITERS):
                for k in range(KCH):
                    load_bmt(t, k)

            auxt = wp.tile([128, 16], F32, tag="auxt")
            nc.sync.dma_start(out=auxt[:], in_=aux[:])

            bias_t = wp.tile([128, T_ITERS * KCH], F32, tag="bias")
            for t in range(T_ITERS):
                for k in range(KCH):
                    nc.vector.memset(
                        bias_t[:, t * KCH + k : t * KCH + k + 1], float(bvals[t, k])
                    )

            def bmat(idx):
                return bmt[:, idx * 128 : (idx + 1) * 128]

            for it in range(T_ITERS):
                for j in range(NTILES):
                    cls = "top" if j == 0 else ("bot" if j == NTILES - 1 else "mid")
                    # 3 gated slots A/B/C per image; GpSimd folds them in
                    # place so no extra sum tiles are needed: A=g0, B=g1, A+=B;
                    # B=g2, C=g3, B+=C, A+=B; update reads A.
                    g = {
                        im: sp.tile([128, 3 * WI], F32, tag=f"g{im}", name=f"g{im}")
                        for im in range(IMGS)
                    }
                    for k in range(KCH):
                        dks = {
                            im: pp.tile([128, WI], F32, tag=f"d{im}", name=f"d{im}")
                            for im in range(IMGS)
                        }
                        passes = []
                        for Dx in DXS:
                            key = (
                                (it, k, Dx, cls)
                                if (it, k, Dx, cls) in bindex
                                else (it, k, Dx, "mid")
                            )
                            # out image col c reads xpad col c+2+Dx
                            passes.append(
                                (bindex[key], 0, WI, slice(2 + Dx, 514 + Dx))
                            )
                        # corrL: out cols (0,1) <- (x col 0, zero pad col 1)
                        passes.append(
                            (bindex[(it, k, "corrL")], 0, 2, slice(2, 0, -1))
                        )
                        # corrR: out cols (510,511) <- (zero pad col 514, x col 511)
                        passes.append(
                            (bindex[(it, k, "corrR")], 510, 512, slice(514, 512, -1))
                        )
                        for pi, (bi, o0, o1, rsl) in enumerate(passes):
                            for im in range(IMGS):
                                nc.tensor.matmul(
                                    dks[im][:, o0:o1],
                                    bmat(bi),
                                    xt[im, j][:, rsl],
                                    start=(pi == 0),
                                    stop=(pi == len(passes) - 1),
                                )
                        slot = [0, 1, 1, 2][k] * WI  # A,B,B,C
                        for im in range(IMGS):
                            ek = sp.tile([128, WI], F32, tag=f"e{im}")
                            nc.scalar.activation(
                                ek[:],
                                dks[im][:],
                                AF.Derivative_Erf,
                                bias=bias_t[:, it * KCH + k : it * KCH + k + 1],
                                scale=1.0,
                            )
                            nc.vector.scalar_tensor_tensor(
                                out=g[im][:, slot : slot + WI],
                                in0=dks[im][:],
                                scalar=float(bvals[it, k]),
                                in1=ek[:],
                                op0=ALU.add,
                                op1=ALU.mult,
                            )
                        if k == 1:
                            for im in range(IMGS):
                                nc.gpsimd.tensor_tensor(
                                    out=g[im][:, 0:WI],
                                    in0=g[im][:, 0:WI],
                                    in1=g[im][:, WI : 2 * WI],
                                    op=ALU.add,
                                )
                        if k == 3:
                            for im in range(IMGS):
                                nc.gpsimd.tensor_tensor(
                                    out=g[im][:, WI : 2 * WI],
                                    in0=g[im][:, WI : 2 * WI],
                                    in1=g[im][:, 2 * WI : 3 * WI],
                                    op=ALU.add,
                                )
                    # mid tiles have no pad rows: constant scalar, no mask
                    # read. Edge tiles mask pad partitions via the aux AP
                    # (DVE partition ranges must start aligned, so a partial
                    # partition range is not an option).
                    edge = j in (0, NTILES - 1)
                    for im in range(IMGS):
                        nc.gpsimd.tensor_tensor(
                            out=g[im][:, 0:WI],
                            in0=g[im][:, 0:WI],
                            in1=g[im][:, WI : 2 * WI],
                            op=ALU.add,
                        )
                        # out keeps the tile's native (f32r) dtype so the DVE
                        # rounds — required for f32r-matmul inputs
                        nc.vector.scalar_tensor_tensor(
                            out=xt[im, j][:, 2:514],
                            in0=g[im][:, 0:WI],
                            scalar=(auxt[:, j : j + 1] if edge else -C_UPD),
                            in1=as_f32(xt[im, j][:, 2:514]),
                            op0=ALU.mult,
                            op1=ALU.add,
                        )
                        if it == T_ITERS - 1:
                            rows = CORE_ROWS[j]
                            nc.sync.dma_start(
                                out=yo[im, 116 * j : 116 * j + rows, :],
                                in_=as_f32(
                                    xt[im, j][CORE_LO : CORE_LO + rows, 2:514]
                                ),
                            )
    nc.compile()
    return nc


_CACHE = {}


def _get_program(Wc, bc):
    key = (Wc.tobytes(), bc.tobytes())
    if key not in _CACHE:
        barr, bindex = _build_bmats(Wc.astype(np.float64))
        # SBUF layout [p, n*128+m]
        bflat = np.ascontiguousarray(
            barr.transpose(1, 0, 2).reshape(128, _NB * 128), dtype=np.float32
        )
        nc = _build_nc(bc.astype(np.float64), bindex)
        _CACHE[key] = (nc, bflat)
    return _CACHE[key]


def _install_trace_shim():
    """The agent image lacks antenv.axon_hooks; rebuild the NTFF hook from
    trn_boot's ctypes recipe and skip the artifact upload."""
    import types

    if "antenv.axon_hooks" in sys.modules:
        return
    try:
        from trn_agent_boot.trn_boot import _ntff_profile_via_ctypes

        hook = _ntff_profile_via_ctypes("/opt/axon/libaxon_pjrt.so")
    except Exception:
        hook = None
    mod = types.ModuleType("antenv.axon_hooks")
    mod.get_axon_ntff_profile_hook = lambda: hook
    mod.set_axon_ntff_profile_hook = lambda h: None
    sys.modules["antenv.axon_hooks"] = mod
    import concourse.bass_utils as bu

    bu.upload_artifacts = lambda d: "local://skipped"


def kernel(x, W, b, _trace=False, _tracedir=None):
    x = np.asarray(x)
    W = np.asarray(W)
    b = np.asarray(b)
    nc, bflat = _get_program(W, b)
    am = np.zeros((128, 16), np.float32)
    am[:, :NTILES] = _build_masks()
    in_maps = []
    for c in range(NCORES):
        shard = np.zeros((IMGS, XROWS, XW), np.float32)
        shard[:, 6 : 6 + H, 2 : 2 + WI] = x[c * IMGS : (c + 1) * IMGS, 0]
        in_maps.append({"xs": shard, "bmat": bflat, "aux": am})
    kw = {}
    if _trace:
        _install_trace_shim()
        kw = {"trace": True, "tmpdir": _tracedir}
    res = run_bass_kernel_spmd(nc, in_maps, list(range(NCORES)), **kw)
    out = np.concatenate([res.results[c]["out"] for c in range(NCORES)], axis=0)
    out = out[:, None].astype(x.dtype)
    kernel._last = res
    return out


# revision 20
# speedup vs baseline: 2.8923x; 1.0590x over previous
"""Trainium2 Bass kernel for nn_Deep_AD_F_58213986730479 (dense_cnn).

Math (per iteration t of 3):
    feats = 4 one-pixel zero-padded shifts (N,S,W,E) of x        [n,4,h,w]
    d     = conv3x3(feats, W[t]) + b[t]                          [n,4,h,w]
    x    -= sum_k d_k * exp(-d_k^2) / 4

Implementation:
  - Pure data parallel: batch 32 -> 8 cores x 4 images.
  - The shift+conv composes into a 21-tap stencil on x. Vertical taps are
    applied with banded-matrix matmuls on TensorE (contraction over image
    rows on partitions); horizontal taps via 5 column-shifted accumulating
    matmuls into PSUM. All matmuls run in float32r (1 cycle/row vs 4 for
    f32). f32r ISA restrictions (even moving counts, 8B-aligned dst) are
    met by padding x tiles to 516 columns with DRAM-staged zeros so every
    Dx pass is a full 512-wide matmul, and by expressing the two column-
    edge corrections as 2-col matmuls whose second lane reads a zero pad
    column via a negative-stride rhs AP.
  - exp(-d^2) comes from one ScalarE op: Derivative_Erf = 2/sqrt(pi)*exp(-x^2);
    the bias add (d+b) rides free in the activation and in the DVE
    scalar_tensor_tensor that forms gated = (d+b)*e. Channel sum on GpSimd,
    final x update is one fused DVE scalar_tensor_tensor (which rounds its
    output to f32r as the BIR verifier requires for f32r-matmul inputs).
  - Each 512x512 image is 5 row-tiles [128,516] (stride 116, 6-row halo);
    3 iterations shrink the valid halo by 2 rows each, so no cross-tile
    traffic is ever needed. Zero pad rows are staged in DRAM (no memsets).
  - Loop order (it, j, k, pass, im) keeps one stationary band matrix for 4
    consecutive matmuls and lets tile j's output DMA overlap tile j+1.
"""
import sys

sys.path.insert(0, "/opt/trn_rl_repo")

import math
import numpy as np

import concourse.bass as bass
import concourse.bacc as bacc
import concourse.mybir as mybir
from concourse.tile import TileContext
from concourse.bass_utils import run_bass_kernel_spmd

F32 = mybir.dt.float32
F32R = mybir.dt.float32r
AF = mybir.ActivationFunctionType
ALU = mybir.AluOpType

NCORES = 8
IMGS = 4          # images per core
H = WI = 512
XW = 516          # x tile width: 2 zero pad cols each side
XROWS = 592       # padded DRAM rows: 6 zero, 512 image, 74 zero
T_ITERS = 3
KCH = 4
NTILES = 5
TSTART = [-6, 110, 226, 342, 458]   # image row held by partition 0 of tile j
CORE_LO = 6                          # first owned partition of each tile
CORE_ROWS = [116, 116, 116, 116, 48]
C_UPD = math.sqrt(math.pi) / 8.0     # 1/4 * sqrt(pi)/2 (Derivative_Erf scale)

# feats channel order in reference: N, S, W, E
OY = [-1, 1, 0, 0]
OX = [0, 0, -1, 1]

DXS = [0, -1, 1, -2, 2]

_MM_DTYPE = __import__("os").environ.get("KERNEL_MM_DTYPE", "f32r")  # f32 | f32r


def _composite_taps(Wc):
    """T[t,k,Dy+2,Dx+2] = sum of W[t,k,i,dy+1,dx+1] with dy+oy_i=Dy, dx+ox_i=Dx."""
    taps = np.zeros((T_ITERS, KCH, 5, 5), np.float64)
    for t in range(T_ITERS):
        for k in range(KCH):
            for i in range(4):
                for dy in (-1, 0, 1):
                    for dx in (-1, 0, 1):
                        taps[t, k, dy + OY[i] + 2, dx + OX[i] + 2] += Wc[
                            t, k, i, dy + 1, dx + 1
                        ]
    return taps


def _build_bmats(Wc):
    """Dense lhsT matrices, returned as array [NB,128,128] f32 plus an index fn.

    Layout per (t,k): 5 generic B_Dx, then 3 top-variant (Dx=-1,0,1), then
    3 bottom-variant, then left corr, right corr = 13 matrices.
    B[in_row, out_row] = tap[in-out, Dx].
    """
    taps = _composite_taps(Wc)
    mats = []
    index = {}

    def band(vals_by_dy):
        B = np.zeros((128, 128), np.float64)
        for dy, v in vals_by_dy.items():
            B += v * np.eye(128, k=-dy)
        return B

    for t in range(T_ITERS):
        for k in range(KCH):
            per_dx = {}
            for Dx in (-2, -1, 0, 1, 2):
                per_dx[Dx] = band(
                    {Dy: taps[t, k, Dy + 2, Dx + 2] for Dy in range(-2, 3)}
                )
            for Dx in DXS:
                index[(t, k, Dx, "mid")] = len(mats)
                mats.append(per_dx[Dx])
            for Dx in (-1, 0, 1):
                Btop = per_dx[Dx].copy()
                # image row 0 = partition CORE_LO of tile 0: remove south-ch dy=-1
                Btop[CORE_LO, CORE_LO] -= Wc[t, k, 1, 0, Dx + 1]
                index[(t, k, Dx, "top")] = len(mats)
                mats.append(Btop)
            for Dx in (-1, 0, 1):
                Bbot = per_dx[Dx].copy()
                # image row 511 = partition 53 of tile 4: remove north-ch dy=+1
                p = (H - 1) - TSTART[4]  # = 53
                Bbot[p, p] -= Wc[t, k, 0, 2, Dx + 1]
                index[(t, k, Dx, "bot")] = len(mats)
                mats.append(Bbot)
            # column-edge corrections (vertical 3-tap bands)
            Bl = band({dy: -Wc[t, k, 3, dy + 1, 0] for dy in (-1, 0, 1)})
            index[(t, k, "corrL")] = len(mats)
            mats.append(Bl)
            Br = band({dy: -Wc[t, k, 2, dy + 1, 2] for dy in (-1, 0, 1)})
            index[(t, k, "corrR")] = len(mats)
            mats.append(Br)
    arr = np.stack(mats).astype(np.float32)
    return arr, index


_NB = T_ITERS * KCH * 13
_PERTK = 13 * 128  # bmat cols per (t,k)


def _build_masks():
    """Per-tile update masks [128, NTILES]: -C_UPD at real image rows, 0 at pad."""
    m = np.full((128, NTILES), -C_UPD, np.float32)
    for j in range(NTILES):
        r0 = TSTART[j]
        plo = max(0, -r0)
        phi = min(128, H - r0)
        m[0:plo, j] = 0.0
        m[phi:128, j] = 0.0
    return m


def _build_nc(bvals, bindex):
    nc = bacc.Bacc(None, target_bir_lowering=False)
    xs = nc.declare_dram_parameter("xs", [IMGS, XROWS, XW], F32, isOutput=False)
    bmw = _NB * 128
    bm = nc.declare_dram_parameter("bmat", [128, bmw], F32, isOutput=False)
    aux = nc.declare_dram_parameter("aux", [128, 16], F32, isOutput=False)
    yo = nc.declare_dram_parameter("out", [IMGS, H, WI], F32, isOutput=True)

    mm_dt = F32R if _MM_DTYPE == "f32r" else F32

    def as_mm(ap):
        # DRAM-side AP relabel so DMA into an f32r tile is cast-free
        return ap.bitcast(F32R) if _MM_DTYPE == "f32r" else ap

    def as_f32(ap):
        # non-PE engines and DMA-out read f32r tiles as plain f32
        return ap.bitcast(F32) if _MM_DTYPE == "f32r" else ap

    with TileContext(nc) as tc:
        with (
            tc.tile_pool(name="wts", bufs=1) as wp,
            tc.tile_pool(name="xdata", bufs=1) as xp,
            tc.tile_pool(name="work", bufs=2) as sp,
            tc.tile_pool(name="ps", bufs=2, space="PSUM") as pp,
        ):
            bmt = wp.tile([128, bmw], mm_dt, tag="bmt")

            def load_bmt(t, k):
                i0 = (t * KCH + k) * _PERTK
                nc.sync.dma_start(
                    out=bmt[:, i0 : i0 + _PERTK],
                    in_=as_mm(bm[:, i0 : i0 + _PERTK]),
                )

            xt = {}

            def load_x(j):
                for im in range(IMGS):
                    tile = xp.tile(
                        [128, XW], mm_dt, tag=f"x{im}_{j}", name=f"x{im}_{j}"
                    )
                    xt[im, j] = tile
                    nc.sync.dma_start(
                        out=tile[:],
                        in_=as_mm(xs[im, TSTART[j] + 6 : TSTART[j] + 134, :]),
                    )

            # DMA order matters: the first tile group needs bmt(t0,k0) and
            # x(*,0) first; everything else streams in behind while the PE
            # is already busy.
            load_bmt(0, 0)
            load_x(0)
            for k in range(1, KCH):
                load_bmt(0, k)
            for j in range(1, NTILES):
                load_x(j)
            for t in range(1, T_ITERS):
                for k in range(KCH):
                    load_bmt(t, k)

            auxt = wp.tile([128, 16], F32, tag="auxt")
            nc.sync.dma_start(out=auxt[:], in_=aux[:])

            bias_t = wp.tile([128, T_ITERS * KCH], F32, tag="bias")
            for t in range(T_ITERS):
                for k in range(KCH):
                    nc.vector.memset(
                        bias_t[:, t * KCH + k : t * KCH + k + 1], float(bvals[t, k])
                    )

            def bmat(idx):
                return bmt[:, idx * 128 : (idx + 1) * 128]

            def flush(pend):
                """Late channel adds + x update (+ final-iter output DMA) for
                a finished group. Deferred into the middle of the next group
                so the in-order DVE/GpSimd queues never stall the PE."""
                if pend is None:
                    return
                p_it, p_j, p_g = pend
                edge = p_j in (0, NTILES - 1)
                for im in range(IMGS):
                    nc.gpsimd.tensor_tensor(
                        out=p_g[im][:, WI : 2 * WI],
                        in0=p_g[im][:, WI : 2 * WI],
                        in1=p_g[im][:, 2 * WI : 3 * WI],
                        op=ALU.add,
                    )
                    nc.gpsimd.tensor_tensor(
                        out=p_g[im][:, 0:WI],
                        in0=p_g[im][:, 0:WI],
                        in1=p_g[im][:, WI : 2 * WI],
                        op=ALU.add,
                    )
                    # out keeps the tile's native (f32r) dtype so the DVE
                    # rounds — required for f32r-matmul inputs. Mid tiles
                    # have no pad rows: constant scalar instead of mask AP.
                    nc.vector.scalar_tensor_tensor(
                        out=xt[im, p_j][:, 2:514],
                        in0=p_g[im][:, 0:WI],
                        scalar=(auxt[:, p_j : p_j + 1] if edge else -C_UPD),
                        in1=as_f32(xt[im, p_j][:, 2:514]),
                        op0=ALU.mult,
                        op1=ALU.add,
                    )
                    if p_it == T_ITERS - 1:
                        rows = CORE_ROWS[p_j]
                        nc.sync.dma_start(
                            out=yo[im, 116 * p_j : 116 * p_j + rows, :],
                            in_=as_f32(
                                xt[im, p_j][CORE_LO : CORE_LO + rows, 2:514]
                            ),
                        )

            pending = None
            for it in range(T_ITERS):
                for j in range(NTILES):
                    cls = "top" if j == 0 else ("bot" if j == NTILES - 1 else "mid")
                    # 3 gated slots A/B/C per image: A=g0, B=g1, A+=B early;
                    # B=g2+g3 and the final A+=B happen in flush() one group
                    # later (g double-buffers via bufs=2).
                    g = {
                        im: sp.tile([128, 3 * WI], F32, tag=f"g{im}", name=f"g{im}")
                        for im in range(IMGS)
                    }
                    for k in range(KCH):
                        dks = {
                            im: pp.tile([128, WI], F32, tag=f"d{im}", name=f"d{im}")
                            for im in range(IMGS)
                        }
                        passes = []
                        for Dx in DXS:
                            key = (
                                (it, k, Dx, cls)
                                if (it, k, Dx, cls) in bindex
                                else (it, k, Dx, "mid")
                            )
                            # out image col c reads xpad col c+2+Dx
                            passes.append(
                                (bindex[key], 0, WI, slice(2 + Dx, 514 + Dx))
                            )
                        # corrL: out cols (0,1) <- (x col 0, zero pad col 1)
                        passes.append(
                            (bindex[(it, k, "corrL")], 0, 2, slice(2, 0, -1))
                        )
                        # corrR: out cols (510,511) <- (zero pad col 514, x col 511)
                        passes.append(
                            (bindex[(it, k, "corrR")], 510, 512, slice(514, 512, -1))
                        )
                        for pi, (bi, o0, o1, rsl) in enumerate(passes):
                            for im in range(IMGS):
                                nc.tensor.matmul(
                                    dks[im][:, o0:o1],
                                    bmat(bi),
                                    xt[im, j][:, rsl],
                                    start=(pi == 0),
                                    stop=(pi == len(passes) - 1),
                                )
                        if k == 2:
                            flush(pending)
                            pending = None
                        slot = [0, 1, 1, 2][k] * WI  # A,B,B,C
                        for im in range(IMGS):
                            ek = sp.tile([128, WI], F32, tag=f"e{im}")
                            nc.scalar.activation(
                                ek[:],
                                dks[im][:],
                                AF.Derivative_Erf,
                                bias=bias_t[:, it * KCH + k : it * KCH + k + 1],
                                scale=1.0,
                            )
                            nc.vector.scalar_tensor_tensor(
                                out=g[im][:, slot : slot + WI],
                                in0=dks[im][:],
                                scalar=float(bvals[it, k]),
                                in1=ek[:],
                                op0=ALU.add,
                                op1=ALU.mult,
                            )
                        if k == 1:
                            for im in range(IMGS):
                                nc.gpsimd.tensor_tensor(
                                    out=g[im][:, 0:WI],
                                    in0=g[im][:, 0:WI],
                                    in1=g[im][:, WI : 2 * WI],
                                    op=ALU.add,
                                )
                    pending = (it, j, g)
            flush(pending)
    nc.compile()
    return nc


_CACHE = {}


def _get_program(Wc, bc):
    key = (Wc.tobytes(), bc.tobytes())
    if key not in _CACHE:
        barr, bindex = _build_bmats(Wc.astype(np.float64))
        # SBUF layout [p, n*128+m]
        bflat = np.ascontiguousarray(
            barr.transpose(1, 0, 2).reshape(128, _NB * 128), dtype=np.float32
        )
        nc = _build_nc(bc.astype(np.float64), bindex)
        _CACHE[key] = (nc, bflat)
    return _CACHE[key]


def _install_trace_shim():
    """The agent image lacks antenv.axon_hooks; rebuild the NTFF hook from
    trn_boot's ctypes recipe and skip the artifact upload."""
    import types

    if "antenv.axon_hooks" in sys.modules:
        return
    try:
        from trn_agent_boot.trn_boot import _ntff_profile_via_ctypes

        hook = _ntff_profile_via_ctypes("/opt/axon/libaxon_pjrt.so")
    except Exception:
        hook = None
    mod = types.ModuleType("antenv.axon_hooks")
    mod.get_axon_ntff_profile_hook = lambda: hook
    mod.set_axon_ntff_profile_hook = lambda h: None
    sys.modules["antenv.axon_hooks"] = mod
    import concourse.bass_utils as bu

    bu.upload_artifacts = lambda d: "local://skipped"


def kernel(x, W, b, _trace=False, _tracedir=None):
    x = np.asarray(x)
    W = np.asarray(W)
    b = np.asarray(b)
    nc, bflat = _get_program(W, b)
    am = np.zeros((128, 16), np.float32)
    am[:, :NTILES] = _build_masks()
    in_maps = []
    for c in range(NCORES):
        shard = np.zeros((IMGS, XROWS, XW), np.float32)
        shard[:, 6 : 6 + H, 2 : 2 + WI] = x[c * IMGS : (c + 1) * IMGS, 0]
        in_maps.append({"xs": shard, "bmat": bflat, "aux": am})
    kw = {}
    if _trace:
        _install_trace_shim()
        kw = {"trace": True, "tmpdir": _tracedir}
    res = run_bass_kernel_spmd(nc, in_maps, list(range(NCORES)), **kw)
    out = np.concatenate([res.results[c]["out"] for c in range(NCORES)], axis=0)
    out = out[:, None].astype(x.dtype)
    kernel._last = res
    return out
